# revision 1
# baseline (speedup 1.0000x reference)
"""Trainium2 Bass kernel for nn_CSATransformer_25778393710760.

Math: the reference module (eval mode) computes
    p   = softmax(wt(w1(x) + w2(c) + bsa), dim=-2);  h = x * p
    A   = softmax(mask_diag(sigmoid(si + sj^T)), -1); colsum = A.sum(1)
    ui  = x * colsum[..., None]
    y   = PFF(ui) + ui;  out = LN(y) * g + b
With the given parameters (all biases zero, ln identity), PFF is positively
homogeneous (relu(c*z) = c*relu(z) for c > 0) and colsum > 0, so
    y = diag(colsum) @ (x + PFF(x))
and LayerNorm cancels the positive per-row scale up to the eps term
(relative effect ~ eps/var * (1 - 1/colsum^2) ~ 1e-8).  Hence
    out = LN(relu(x @ pfn_w1) @ pfn_w2 + x) * ln_g + ln_b
to well below f32 noise (verified 4.5e-6 max rel err vs the f32 reference,
identical to the reference's own f32-vs-f64 noise floor).

Sharding: pure data parallel over batch B=8 across the 8 NeuronCores.

Kernel layout per core (one batch example, L=4096 rows of D=128):
8 slabs of 512 rows, fully streaming:
  DMA in -> PE transpose to (d,l) -> w1 matmul + relu -> w2 matmul +
  residual add -> PE transpose back -> bn_stats/bn_aggr LN stats ->
  normalize (DVE/ACT split) -> DMA out.
DMA placement matters: slab-0 per-chunk on the two HWDGE rings, bulk
loads throttled (pool bufs) on the gpsimd SWDGE ring so they do not
steal SDMA bandwidth/queue service from the pipeline-filling loads.
"""

import os
import numpy as np

B, L, DX = 8, 4096, 128
_SLABS = 8          # 512-row slabs per core
_CPS = 4            # 128-row chunks per slab

_prog_cache = {}


def _build_program(f32r_mode=False):
    import concourse.tile as tile
    from concourse import bacc, mybir
    from concourse.bass import ts

    f32 = mybir.dt.float32
    f32r = mybir.dt.float32r
    AF = mybir.ActivationFunctionType
    OP = mybir.AluOpType

    nc = bacc.Bacc(None, target_bir_lowering=False)
    x = nc.dram_tensor("x", [L, DX], f32, kind="ExternalInput")
    w1 = nc.dram_tensor("w1", [DX, DX], f32, kind="ExternalInput")
    w2 = nc.dram_tensor("w2", [DX, DX], f32, kind="ExternalInput")
    identp = nc.dram_tensor("identp", [DX, DX + 1], f32, kind="ExternalInput")
    y = nc.dram_tensor("y", [L, DX], f32, kind="ExternalOutput")

    with tile.TileContext(nc) as tc:
        with (
            tc.tile_pool(name="consts", bufs=1) as consts,
            tc.tile_pool(name="io", bufs=3) as io,
            tc.tile_pool(name="work", bufs=3) as work,
            tc.tile_pool(name="small", bufs=4) as small,
            tc.tile_pool(name="xg_pool", bufs=2) as xg_pool,
            tc.tile_pool(name="ps_t", bufs=2, space="PSUM") as ps_t,
            tc.tile_pool(name="ps_mm", bufs=3, space="PSUM") as ps_mm,
            tc.tile_pool(name="ps_out", bufs=3, space="PSUM") as ps_out,
        ):
            # ---- tiny const DMAs first: transposes gate on ident ----
            identp_sb = consts.tile([128, 129], f32)
            nc.sync.dma_start(out=identp_sb, in_=identp[:, :])
            ident = identp_sb[:, 0:128]

            # ---- issue all x loads up front so slab 0 lands ASAP ----
            # slab 0 loads per-chunk on both HWDGE rings for fastest start;
            # later slabs alternate rings (sync / scalar issue FIFO per ring)
            xgs = []
            chunks0 = []
            for c in range(_CPS):
                xc = xg_pool.tile([128, 128], f32, tag=f"xg0_{c}")
                eng = nc.sync if c % 2 == 0 else nc.scalar
                eng.dma_start(out=xc, in_=x[ts(c, 128), :])
                chunks0.append(xc)
            xgs.append(chunks0)
            w1_sb = consts.tile([128, 128], f32)
            w2_sb = consts.tile([128, 128], f32)
            nc.sync.dma_start(out=w1_sb, in_=w1[:, :])
            nc.scalar.dma_start(out=w2_sb, in_=w2[:, :])
            # bulk loads ride the idle gpsimd SWDGE ring so the sync/scalar
            # queues stay short (their EVSEMs gate the first transposes)
            for g in range(1, _SLABS):
                xg = xg_pool.tile([128, _CPS, 128], f32, tag="xg")
                src = x[ts(g, 512), :].rearrange("(c p) d -> p c d", p=128)
                nc.gpsimd.dma_start(out=xg, in_=src)
                xgs.append(xg)
            if f32r_mode:
                w1_r = consts.tile([128, 128], f32r)
                w2_r = consts.tile([128, 128], f32r)
                nc.scalar.copy(out=w1_r, in_=w1_sb)
                nc.scalar.copy(out=w2_r, in_=w2_sb)
                w1_mm, w2_mm = w1_r, w2_r
            else:
                w1_mm, w2_mm = w1_sb, w2_sb
            eps = consts.tile([128, 1], f32)
            nc.vector.memset(eps, 1e-6)
            # spin the PE on dummy transposes while waiting for x DMAs:
            # ~4us of sustained activity flips the HAM clock gate to 2.4GHz
            # before the real matmuls start (cold fp32 matmuls run at half
            # rate)
            pewarm = ps_t.tile([128, _CPS, 128], f32, tag="xtp")
            for _ in range(18):
                nc.tensor.transpose(pewarm[:, 0, :], ident, ident)
            warmsink = consts.tile([128, 1], f32)
            nc.vector.tensor_copy(out=warmsink, in_=pewarm[:, 0, 0:1])
            # warm up the ACT table sets off the critical path
            warm = consts.tile([128, 1], f32)
            nc.scalar.activation(out=warm, in_=eps, func=AF.Relu)
            nc.scalar.activation(out=warm, in_=eps, func=AF.Sqrt)
            nc.scalar.activation(out=warm, in_=eps, func=AF.Identity, bias=eps)

            for g in range(_SLABS):
                # ---- transpose to (d, l) layout ----
                xtp = ps_t.tile([128, _CPS, 128], f32, tag="xtp")
                for c in range(_CPS):
                    xin = xgs[g][c] if g == 0 else xgs[g][:, c, :]
                    nc.tensor.transpose(xtp[:, c, :], xin, ident)
                xT = work.tile([128, _CPS, 128], f32r if f32r_mode else f32,
                               tag="xT")
                nc.scalar.copy(out=xT, in_=xtp)
                xT2 = xT.rearrange("p c d -> p (c d)")
                xT2f = xT2.bitcast(f32) if f32r_mode else xT2

                # ---- PFF: y1T = relu(w1T @ xT); PT = w2T @ y1T + xT ----
                y1p = ps_mm.tile([128, 512], f32, tag="mm")
                nc.tensor.matmul(y1p, lhsT=w1_mm, rhs=xT2, start=True, stop=True)
                y1s = work.tile([128, 512], f32r if f32r_mode else f32, tag="y1s")
                nc.scalar.activation(out=y1s, in_=y1p, func=AF.Relu)
                pp = ps_mm.tile([128, 512], f32, tag="mm")
                nc.tensor.matmul(pp, lhsT=w2_mm, rhs=y1s, start=True, stop=True)
                pt = work.tile([128, 512], f32, tag="pt")
                nc.vector.tensor_add(out=pt, in0=pp, in1=xT2f)

                # ---- transpose back to (l, d) layout ----
                pn = ps_out.tile([128, _CPS, 128], f32, tag="pn")
                for c in range(_CPS):
                    nc.tensor.transpose(pn[:, c, :], pt[:, ts(c, 128)], ident)

                # ---- LN stats via bn_stats/bn_aggr per chunk ----
                bstats = small.tile([128, _CPS, 6], f32, tag="bstats")
                for c in range(_CPS):
                    nc.vector.bn_stats(out=bstats[:, c, :], in_=pn[:, c, :])
                mv = small.tile([128, _CPS, 2], f32, tag="mv")
                for c in range(_CPS):
                    nc.vector.bn_aggr(out=mv[:, c, :], in_=bstats[:, c, :])

                # rstd = 1/sqrt(var + eps); nmr = -mean * rstd
                # per-half so chunks 0-1 can normalize before 2-3 aggregate
                std = small.tile([128, _CPS], f32, tag="std")
                rstd = small.tile([128, _CPS], f32, tag="rstd")
                nmr = small.tile([128, _CPS], f32, tag="nmr")
                for hh in range(2):
                    hsl = slice(2 * hh, 2 * hh + 2)
                    nc.scalar.activation(
                        out=std[:, hsl], in_=mv[:, hsl, 1], func=AF.Sqrt,
                        scale=1.0, bias=eps,
                    )
                    nc.vector.reciprocal(out=rstd[:, hsl], in_=std[:, hsl])
                    nc.vector.scalar_tensor_tensor(
                        out=nmr[:, hsl], in0=mv[:, hsl, 0], scalar=-1.0,
                        in1=rstd[:, hsl], op0=OP.mult, op1=OP.mult,
                    )

                # ---- apply LN from PSUM: out = pn * rstd + nmr ----
                og = io.tile([128, _CPS, 128], f32, tag="og")
                for c in range(_CPS):
                    if c % 2 == 0:
                        nc.vector.tensor_scalar(
                            out=og[:, c, :], in0=pn[:, c, :],
                            scalar1=rstd[:, c : c + 1], scalar2=nmr[:, c : c + 1],
                            op0=OP.mult, op1=OP.add,
                        )
                    else:
                        nc.scalar.activation(
                            out=og[:, c, :], in_=pn[:, c, :], func=AF.Identity,
                            bias=nmr[:, c : c + 1], scale=rstd[:, c : c + 1],
                        )

                for h in range(2):
                    dst = y[ts(2 * g + h, 256), :].rearrange(
                        "(c p) d -> p c d", p=128
                    )
                    nc.sync.dma_start(out=dst, in_=og[:, 2 * h : 2 * h + 2, :])
    nc.finalize()
    return nc


def _ensure_ntff_hook():
    """Register the axon NTFF profiling hook if the image lacks antenv.axon_hooks."""
    try:
        from antenv.axon_hooks import get_axon_ntff_profile_hook  # noqa: F401
        return
    except ImportError:
        pass
    import sys
    import types

    import antenv
    from trn_agent_boot.trn_boot import _ntff_profile_via_ctypes

    hook = _ntff_profile_via_ctypes("/opt/axon/libaxon_pjrt.so")
    mod = types.ModuleType("antenv.axon_hooks")
    mod._hook = hook
    mod.set_axon_ntff_profile_hook = lambda h: setattr(mod, "_hook", h)
    mod.get_axon_ntff_profile_hook = lambda: mod._hook
    sys.modules["antenv.axon_hooks"] = mod
    antenv.axon_hooks = mod


def _run_device(x, w1, w2, trace=False):
    import concourse.bass_utils as bass_utils
    from concourse.bass_utils import run_bass_kernel_spmd

    if trace:
        try:
            _ensure_ntff_hook()
            bass_utils.upload_artifacts = lambda tmpdir: str(tmpdir)
        except Exception as e:  # profiling is best-effort
            print(f"ntff hook unavailable ({e}); running without trace")
            trace = False

    f32r_mode = bool(int(os.environ.get("CSA_F32R", "0")))
    key = ("prog", f32r_mode)
    if key not in _prog_cache:
        _prog_cache[key] = _build_program(f32r_mode)
    nc = _prog_cache[key]
    if f32r_mode:
        # pre-round x on the host to the fp32r grid so the on-device f32r
        # rounding of xT is lossless (keeps residual consistent)
        xi = np.ascontiguousarray(x, dtype=np.float32).view(np.uint32)
        x = (xi & np.uint32(0xFFFFF000)).view(np.float32).reshape(x.shape)
    w1c = np.ascontiguousarray(w1, dtype=np.float32)
    w2c = np.ascontiguousarray(w2, dtype=np.float32)
    identp = np.concatenate(
        [np.eye(DX, dtype=np.float32), np.ones((DX, 1), np.float32)], axis=1
    )
    in_maps = [
        {
            "x": np.ascontiguousarray(x[b], dtype=np.float32),
            "w1": w1c,
            "w2": w2c,
            "identp": identp,
        }
        for b in range(B)
    ]
    res = run_bass_kernel_spmd(
        nc, in_maps, core_ids=list(range(B)), trace=trace,
        trace_cores=list(range(B)) if trace else None,
    )
    kernel.last_result = res
    kernel.last_exec_time_ns = res.exec_time_ns
    return np.stack([r["y"] for r in res.results], axis=0)


def _numpy_fallback(inputs):
    """Faithful (but slow) mirror of the reference for unexpected inputs."""
    f32 = np.float32
    x = np.asarray(inputs["x"], f32)
    c = np.asarray(inputs["c"], f32)
    W1 = np.asarray(inputs["W1"], f32); W2 = np.asarray(inputs["W2"], f32)
    wt_w = np.asarray(inputs["wt_w"], f32); bsa = np.asarray(inputs["bsa"], f32)
    Wsa1 = np.asarray(inputs["Wsa1"], f32); Wsa2 = np.asarray(inputs["Wsa2"], f32)
    wsat_w = np.asarray(inputs["wsat_w"], f32)
    wsat_b = np.asarray(inputs["wsat_b"], f32); bsa1 = np.asarray(inputs["bsa1"], f32)
    pfn_w1 = np.asarray(inputs["pfn_w1"], f32); pfn_b1 = np.asarray(inputs["pfn_b1"], f32)
    pfn_w2 = np.asarray(inputs["pfn_w2"], f32); pfn_b2 = np.asarray(inputs["pfn_b2"], f32)
    ln_g = np.asarray(inputs["ln_g"], f32); ln_b = np.asarray(inputs["ln_b"], f32)
    Bs, Ls, _ = x.shape
    wx = x @ W1
    wq = c @ W2
    logits = (wx + wq[:, None, :] + bsa) @ wt_w
    m = logits.max(-1, keepdims=True)
    e = np.exp(logits - m)
    p = (e / e.sum(-1, keepdims=True))[..., None]
    h = x * p
    si = (h @ Wsa1) @ wsat_w
    sj = (h @ Wsa2) @ wsat_w
    const = bsa1 @ wsat_w + wsat_b
    colsum = np.zeros((Bs, Ls), f32)
    blk = 512
    for b in range(Bs):
        for i0 in range(0, Ls, blk):
            s = 1.0 / (1.0 + np.exp(-(si[b, i0 : i0 + blk, None] + sj[b, None, :] + const)))
            for r in range(s.shape[0]):
                s[r, i0 + r] = -np.inf
            sm = s.max(-1, keepdims=True)
            ee = np.exp(s - sm)
            colsum[b] += (ee / ee.sum(-1, keepdims=True)).sum(0)
    ui = x * colsum[..., None]
    yv = np.maximum(ui @ pfn_w1 + pfn_b1, 0.0)
    yv = yv @ pfn_w2 + pfn_b2 + ui
    mu = yv.mean(-1, keepdims=True)
    var = ((yv - mu) ** 2).mean(-1, keepdims=True)
    return ((yv - mu) / np.sqrt(var + 1e-6) * ln_g + ln_b).astype(f32)


def kernel(**inputs):
    x = np.asarray(inputs["x"], dtype=np.float32)
    pfn_w1 = np.asarray(inputs["pfn_w1"], dtype=np.float32)
    pfn_w2 = np.asarray(inputs["pfn_w2"], dtype=np.float32)

    fast_ok = (
        x.shape == (B, L, DX)
        and not np.any(np.asarray(inputs["pfn_b1"]))
        and not np.any(np.asarray(inputs["pfn_b2"]))
        and np.all(np.asarray(inputs["ln_g"]) == 1.0)
        and not np.any(np.asarray(inputs["ln_b"]))
    )
    if not fast_ok:
        return _numpy_fallback(inputs)

    trace = bool(int(os.environ.get("CSA_TRACE", "0")))
    return _run_device(x, pfn_w1, pfn_w2, trace=trace)


kernel.last_exec_time_ns = None
kernel.last_result = None



# revision 7
# speedup vs baseline: 1.0530x; 1.0530x over previous
"""Trainium2 Bass kernel for nn_CSATransformer_25778393710760.

Math: the reference module (eval mode) computes
    p   = softmax(wt(w1(x) + w2(c) + bsa), dim=-2);  h = x * p
    A   = softmax(mask_diag(sigmoid(si + sj^T)), -1); colsum = A.sum(1)
    ui  = x * colsum[..., None]
    y   = PFF(ui) + ui;  out = LN(y) * g + b
With the given parameters (all biases zero, ln identity), PFF is positively
homogeneous (relu(c*z) = c*relu(z) for c > 0) and colsum > 0, so
    y = diag(colsum) @ (x + PFF(x))
and LayerNorm cancels the positive per-row scale up to the eps term
(relative effect ~ eps/var * (1 - 1/colsum^2) ~ 1e-8).  Hence
    out = LN(relu(x @ pfn_w1) @ pfn_w2 + x) * ln_g + ln_b
to well below f32 noise.

Sharding: pure data parallel over batch B=8 across the 8 NeuronCores.

Kernel layout per core (one batch example, L=4096 rows of D=128), all
matmul-path data in fp16 (input rounding ~5e-4 rel, far under the 2e-2
gate), 8 slabs of 512 rows, fully streaming:
  SWDGE cast-DMA in (f32 HBM -> f16 SBUF, 2KB/partition lines) ->
  PE transpose (f16 PSUM) -> DVE copy to SBUF -> w1 matmul -> ACT relu
  -> w2 matmul + residual folded in as a second accumulating matmul
  (ident @ xT, start=False) -> ACT copy -> PE transpose back (f16 PSUM)
  -> ACT copy to SBUF -> per-chunk bn_stats/bn_aggr -> sqrt/recip ->
  normalize on DVE from SBUF f16 -> HWDGE store (2KB lines).
fp16 keeps the PE at 1 cycle/col (fp32 is 4) and unlocks the DVE 2x/4x
packed modes; partition p holds rows 4p..4p+3 so every DMA descriptor
line is 2KB contiguous HBM (the transpose column permutation this
induces cancels between the in- and out-transposes).
"""

import os
import numpy as np

B, L, DX = 8, 4096, 128
_SLABS = 8          # 512-row slabs per core
_CPS = 4            # 128-row chunks per slab
_WARM_MMS = 28      # PE HAM warmup matmuls (~3us busy)

_prog_cache = {}


def _build_program():
    import concourse.tile as tile
    from concourse import bacc, mybir
    from concourse.bass import ts

    f32 = mybir.dt.float32
    f16 = mybir.dt.float16
    AF = mybir.ActivationFunctionType
    OP = mybir.AluOpType

    nc = bacc.Bacc(None, target_bir_lowering=False)
    x = nc.dram_tensor("x", [L, DX], f32, kind="ExternalInput")
    w1 = nc.dram_tensor("w1", [DX, DX], f16, kind="ExternalInput")
    w2 = nc.dram_tensor("w2", [DX, DX], f16, kind="ExternalInput")
    identp = nc.dram_tensor("identp", [DX, DX], f16, kind="ExternalInput")
    y = nc.dram_tensor("y", [L, DX], f32, kind="ExternalOutput")

    with tile.TileContext(nc) as tc:
        with (
            tc.tile_pool(name="consts", bufs=1) as consts,
            tc.tile_pool(name="xg_pool", bufs=_SLABS) as xg_pool,
            tc.tile_pool(name="work", bufs=2) as work,
            tc.tile_pool(name="pnp", bufs=3) as pnp,
            tc.tile_pool(name="small", bufs=4) as small,
            tc.tile_pool(name="io", bufs=3) as io,
            tc.tile_pool(name="ps_t", bufs=2, space="PSUM") as ps_t,
            tc.tile_pool(name="ps_mm1", bufs=2, space="PSUM") as ps_mm1,
            tc.tile_pool(name="ps_mm2", bufs=2, space="PSUM") as ps_mm2,
            tc.tile_pool(name="ps_tb", bufs=2, space="PSUM") as ps_tb,
        ):
            # ---- tiny const DMAs first: everything gates on these ----
            ident_sb = consts.tile([128, 128], f16)
            w1_sb = consts.tile([128, 128], f16)
            w2_sb = consts.tile([128, 128], f16)
            nc.sync.dma_start(out=ident_sb, in_=identp[:, :])
            nc.scalar.dma_start(out=w1_sb, in_=w1[:, :])
            nc.scalar.dma_start(out=w2_sb, in_=w2[:, :])
            eps = consts.tile([128, 1], f32)
            nc.vector.memset(eps, 1e-6)

            # ---- issue all x cast-loads (f32 HBM -> f16 SBUF, SWDGE).
            # slab 0 per-chunk so the first transpose can start ASAP.
            xgs = []
            for g in range(_SLABS):
                xg = xg_pool.tile([128, _CPS, 128], f16, tag="xg")
                src = x[ts(g, 512), :].rearrange("(p r) d -> p r d", p=128)
                if g == 0:
                    for c in range(_CPS):
                        nc.gpsimd.dma_start(out=xg[:, c, :], in_=src[:, c, :])
                else:
                    nc.gpsimd.dma_start(out=xg, in_=src)
                xgs.append(xg)

            # ---- PE HAM warmup: sustained matmul activity flips the
            # clock gate to 2.4GHz before the real pipeline fills.
            warm_ps = ps_mm1.tile([128, 512], f32, tag="mm1")
            for _ in range(_WARM_MMS):
                nc.tensor.matmul(
                    warm_ps[:, 0:128], lhsT=ident_sb, rhs=ident_sb,
                    start=True, stop=True,
                )
            warmsink = consts.tile([128, 1], f32)
            nc.vector.tensor_copy(out=warmsink, in_=warm_ps[:, 0:1])
            # warm the ACT table set (rsqrt anchor; relu/identity ride along)
            warm = consts.tile([128, 1], f32)
            nc.scalar.activation(out=warm, in_=eps, func=AF.Sqrt)
            nc.scalar.activation(out=warm, in_=eps, func=AF.Relu)
            nc.scalar.activation(out=warm, in_=eps, func=AF.Identity, bias=eps)

            for g in range(_SLABS):
                xg = xgs[g]

                # ---- transpose to (d, l) layout; f16 PSUM out ----
                xtp = ps_t.tile([128, _CPS, 128], f16, tag="xtp")
                for c in range(_CPS):
                    nc.tensor.transpose(xtp[:, c, :], xg[:, c, :], ident_sb)
                xT = work.tile([128, 512], f16, tag="xT")
                nc.vector.tensor_copy(out=xT, in_=xtp.rearrange("p c d -> p (c d)"))

                # ---- PFF in transposed domain; residual folded into the
                # mm2 accumulation group as ident.T @ xT ----
                y1p = ps_mm1.tile([128, 512], f32, tag="mm1")
                nc.tensor.matmul(y1p, lhsT=w1_sb, rhs=xT, start=True, stop=True)
                y1s = work.tile([128, 512], f16, tag="y1s")
                nc.scalar.activation(out=y1s, in_=y1p, func=AF.Relu)
                pp = ps_mm2.tile([128, 512], f32, tag="mm2")
                nc.tensor.matmul(pp, lhsT=w2_sb, rhs=y1s, start=True, stop=False)
                nc.tensor.matmul(pp, lhsT=ident_sb, rhs=xT, start=False, stop=True)
                y2s = work.tile([128, 512], f16, tag="y2s")
                nc.scalar.activation(out=y2s, in_=pp, func=AF.Identity)

                # ---- transpose back to (l, d); f16 PSUM out ----
                ppT = ps_tb.tile([128, _CPS, 128], f16, tag="ppT")
                for c in range(_CPS):
                    nc.tensor.transpose(ppT[:, c, :], y2s[:, ts(c, 128)], ident_sb)
                pn = pnp.tile([128, _CPS, 128], f16, tag="pn")
                nc.scalar.activation(
                    out=pn.rearrange("p c d -> p (c d)"),
                    in_=ppT.rearrange("p c d -> p (c d)"),
                    func=AF.Identity,
                )

                # ---- LN stats: per-chunk bn_stats + bn_aggr ----
                bstats = small.tile([128, _CPS, 6], f32, tag="bstats")
                for c in range(_CPS):
                    nc.vector.bn_stats(out=bstats[:, c, :], in_=pn[:, c, :])
                mv = small.tile([128, _CPS, 2], f32, tag="mv")
                for c in range(_CPS):
                    nc.vector.bn_aggr(out=mv[:, c, :], in_=bstats[:, c, :])

                # rstd = 1/sqrt(var + eps); nmr = -mean * rstd
                std = small.tile([128, _CPS], f32, tag="std")
                nc.scalar.activation(
                    out=std, in_=mv[:, :, 1], func=AF.Sqrt, scale=1.0, bias=eps,
                )
                rstd = small.tile([128, _CPS], f32, tag="rstd")
                nc.vector.reciprocal(out=rstd, in_=std)
                nmr = small.tile([128, _CPS], f32, tag="nmr")
                nc.vector.scalar_tensor_tensor(
                    out=nmr, in0=mv[:, :, 0], scalar=-1.0, in1=rstd,
                    op0=OP.mult, op1=OP.mult,
                )

                # ---- apply LN on DVE from SBUF f16: out = pn*rstd + nmr ----
                og = io.tile([128, _CPS, 128], f32, tag="og")
                for c in range(_CPS):
                    nc.vector.tensor_scalar(
                        out=og[:, c, :], in0=pn[:, c, :],
                        scalar1=rstd[:, c : c + 1], scalar2=nmr[:, c : c + 1],
                        op0=OP.mult, op1=OP.add,
                    )

                dst = y[ts(g, 512), :].rearrange("(p r) d -> p r d", p=128)
                eng = nc.sync if g % 2 == 0 else nc.scalar
                eng.dma_start(out=dst, in_=og)
    nc.finalize()
    return nc


def _ensure_ntff_hook():
    """Register the axon NTFF profiling hook if the image lacks antenv.axon_hooks."""
    try:
        from antenv.axon_hooks import get_axon_ntff_profile_hook  # noqa: F401
        return
    except ImportError:
        pass
    import sys
    import types

    import antenv
    from trn_agent_boot.trn_boot import _ntff_profile_via_ctypes

    hook = _ntff_profile_via_ctypes("/opt/axon/libaxon_pjrt.so")
    mod = types.ModuleType("antenv.axon_hooks")
    mod._hook = hook
    mod.set_axon_ntff_profile_hook = lambda h: setattr(mod, "_hook", h)
    mod.get_axon_ntff_profile_hook = lambda: mod._hook
    sys.modules["antenv.axon_hooks"] = mod
    antenv.axon_hooks = mod


def _run_device(x, w1, w2, trace=False):
    import concourse.bass_utils as bass_utils
    from concourse.bass_utils import run_bass_kernel_spmd

    if trace:
        try:
            _ensure_ntff_hook()
            bass_utils.upload_artifacts = lambda tmpdir: str(tmpdir)
        except Exception as e:  # profiling is best-effort
            print(f"ntff hook unavailable ({e}); running without trace")
            trace = False

    if "prog" not in _prog_cache:
        _prog_cache["prog"] = _build_program()
    nc = _prog_cache["prog"]
    w1h = np.ascontiguousarray(w1, dtype=np.float16)
    w2h = np.ascontiguousarray(w2, dtype=np.float16)
    identh = np.eye(DX, dtype=np.float16)
    in_maps = [
        {
            "x": np.ascontiguousarray(x[b], dtype=np.float32),
            "w1": w1h,
            "w2": w2h,
            "identp": identh,
        }
        for b in range(B)
    ]
    res = run_bass_kernel_spmd(
        nc, in_maps, core_ids=list(range(B)), trace=trace,
        trace_cores=list(range(B)) if trace else None,
    )
    kernel.last_result = res
    kernel.last_exec_time_ns = res.exec_time_ns
    return np.stack([r["y"] for r in res.results], axis=0)


def _numpy_fallback(inputs):
    """Faithful (but slow) mirror of the reference for unexpected inputs."""
    f32 = np.float32
    x = np.asarray(inputs["x"], f32)
    c = np.asarray(inputs["c"], f32)
    W1 = np.asarray(inputs["W1"], f32); W2 = np.asarray(inputs["W2"], f32)
    wt_w = np.asarray(inputs["wt_w"], f32); bsa = np.asarray(inputs["bsa"], f32)
    Wsa1 = np.asarray(inputs["Wsa1"], f32); Wsa2 = np.asarray(inputs["Wsa2"], f32)
    wsat_w = np.asarray(inputs["wsat_w"], f32)
    wsat_b = np.asarray(inputs["wsat_b"], f32); bsa1 = np.asarray(inputs["bsa1"], f32)
    pfn_w1 = np.asarray(inputs["pfn_w1"], f32); pfn_b1 = np.asarray(inputs["pfn_b1"], f32)
    pfn_w2 = np.asarray(inputs["pfn_w2"], f32); pfn_b2 = np.asarray(inputs["pfn_b2"], f32)
    ln_g = np.asarray(inputs["ln_g"], f32); ln_b = np.asarray(inputs["ln_b"], f32)
    Bs, Ls, _ = x.shape
    wx = x @ W1
    wq = c @ W2
    logits = (wx + wq[:, None, :] + bsa) @ wt_w
    m = logits.max(-1, keepdims=True)
    e = np.exp(logits - m)
    p = (e / e.sum(-1, keepdims=True))[..., None]
    h = x * p
    si = (h @ Wsa1) @ wsat_w
    sj = (h @ Wsa2) @ wsat_w
    const = bsa1 @ wsat_w + wsat_b
    colsum = np.zeros((Bs, Ls), f32)
    blk = 512
    for b in range(Bs):
        for i0 in range(0, Ls, blk):
            s = 1.0 / (1.0 + np.exp(-(si[b, i0 : i0 + blk, None] + sj[b, None, :] + const)))
            for r in range(s.shape[0]):
                s[r, i0 + r] = -np.inf
            sm = s.max(-1, keepdims=True)
            ee = np.exp(s - sm)
            colsum[b] += (ee / ee.sum(-1, keepdims=True)).sum(0)
    ui = x * colsum[..., None]
    yv = np.maximum(ui @ pfn_w1 + pfn_b1, 0.0)
    yv = yv @ pfn_w2 + pfn_b2 + ui
    mu = yv.mean(-1, keepdims=True)
    var = ((yv - mu) ** 2).mean(-1, keepdims=True)
    return ((yv - mu) / np.sqrt(var + 1e-6) * ln_g + ln_b).astype(f32)


def kernel(**inputs):
    x = np.asarray(inputs["x"], dtype=np.float32)
    pfn_w1 = np.asarray(inputs["pfn_w1"], dtype=np.float32)
    pfn_w2 = np.asarray(inputs["pfn_w2"], dtype=np.float32)

    fast_ok = (
        x.shape == (B, L, DX)
        and not np.any(np.asarray(inputs["pfn_b1"]))
        and not np.any(np.asarray(inputs["pfn_b2"]))
        and np.all(np.asarray(inputs["ln_g"]) == 1.0)
        and not np.any(np.asarray(inputs["ln_b"]))
    )
    if not fast_ok:
        return _numpy_fallback(inputs)

    trace = bool(int(os.environ.get("CSA_TRACE", "0")))
    return _run_device(x, pfn_w1, pfn_w2, trace=trace)


kernel.last_exec_time_ns = None
kernel.last_result = None


# revision 9
# speedup vs baseline: 1.0637x; 1.0102x over previous
"""Trainium2 Bass kernel for nn_CSATransformer_25778393710760.

Math: with this problem's parameters (all biases zero, ln affine identity),
the attention colsum scale cancels through LayerNorm (PFF is positively
homogeneous, colsum > 0), so
    out = LN(relu(x @ pfn_w1) @ pfn_w2 + x)
to ~1e-8.  Sharding: pure data parallel over batch B=8 across 8 cores.

Per-core kernel (L=4096 rows, D=128), fp16 matmul path (~5e-4 rel err,
gate is 2e-2), processed as 4 pairs of 1024 rows:
  SWDGE cast-DMA in (f32 HBM -> f16 SBUF, 4KB/partition lines; partition
  p holds rows 8p..8p+7) -> 8x PE transpose (f16 PSUM, LDW-bound) ->
  one DVE copy [128,1024] -> 2x w1 matmul -> one ACT relu [128,1024] ->
  2x (w2 matmul + residual as accumulating ident matmul) -> one ACT copy
  -> 8x PE transpose back (f16 PSUM) -> one DVE copy -> bn_stats +
  wide [128,8] stat combines -> sqrt/recip -> normalize split across
  ACT/DVE/GPSIMD -> HWDGE store (4KB lines).
The transpose column permutation induced by the DMA layout cancels
between the in- and out-transposes.  PE HAM warmup: 9 x N=512 matmuls
(~3.8us sustained) flip the clock gate to 2.4GHz before the pipeline.
"""

import os
import numpy as np

B, L, DX = 8, 4096, 128
_PAIRS = 4          # 1024-row blocks per core
_CPP = 8            # 128-row chunks per block
_WARM_MMS = 9       # N=512 PE warmup matmuls (~3.8us cold)
_BN3D = False       # grouped bn_stats crashes walrus (AP flattens); per chunk
# normalize chunk -> engine: 3x ACT, 2x DVE, 3x GPSIMD
_NORM_ENG = ("act", "act", "act", "dve", "dve", "gps", "gps", "gps")

_prog_cache = {}


def _build_program():
    import concourse.tile as tile
    from concourse import bacc, mybir
    from concourse.bass import ts

    f32 = mybir.dt.float32
    f16 = mybir.dt.float16
    AF = mybir.ActivationFunctionType
    OP = mybir.AluOpType

    nc = bacc.Bacc(None, target_bir_lowering=False)
    x = nc.dram_tensor("x", [L, DX], f32, kind="ExternalInput")
    w1 = nc.dram_tensor("w1", [DX, DX], f16, kind="ExternalInput")
    w2 = nc.dram_tensor("w2", [DX, DX], f16, kind="ExternalInput")
    identp = nc.dram_tensor("identp", [DX, DX], f16, kind="ExternalInput")
    y = nc.dram_tensor("y", [L, DX], f32, kind="ExternalOutput")

    with tile.TileContext(nc) as tc:
        with (
            tc.tile_pool(name="consts", bufs=1) as consts,
            tc.tile_pool(name="xg_pool", bufs=_PAIRS) as xg_pool,
            tc.tile_pool(name="work", bufs=2) as work,
            tc.tile_pool(name="pnp", bufs=2) as pnp,
            tc.tile_pool(name="small", bufs=3) as small,
            tc.tile_pool(name="io", bufs=3) as io,
            tc.tile_pool(name="ps_t", bufs=2, space="PSUM") as ps_t,
            tc.tile_pool(name="ps_mm1", bufs=1, space="PSUM") as ps_mm1,
            tc.tile_pool(name="ps_mm2", bufs=1, space="PSUM") as ps_mm2,
            tc.tile_pool(name="ps_tb", bufs=2, space="PSUM") as ps_tb,
        ):
            # ---- tiny const DMAs first: everything gates on these ----
            ident_sb = consts.tile([128, 128], f16)
            w1_sb = consts.tile([128, 128], f16)
            w2_sb = consts.tile([128, 128], f16)
            nc.sync.dma_start(out=ident_sb, in_=identp[:, :])
            nc.scalar.dma_start(out=w1_sb, in_=w1[:, :])
            nc.scalar.dma_start(out=w2_sb, in_=w2[:, :])
            eps = consts.tile([128, 1], f32)
            nc.vector.memset(eps, 1e-6)
            warm_rhs = consts.tile([128, 512], f16)
            nc.vector.memset(warm_rhs, 0.5)

            # ---- issue all x cast-loads (f32 HBM -> f16 SBUF, SWDGE).
            # pair 0 in quarters so the first transposes start ASAP.
            xgs = []
            for g in range(_PAIRS):
                xg = xg_pool.tile([128, _CPP, 128], f16, tag="xg")
                src = x[ts(g, 1024), :].rearrange("(p r) d -> p r d", p=128)
                if g == 0:
                    for q in range(4):
                        nc.gpsimd.dma_start(
                            out=xg[:, ts(q, 2), :], in_=src[:, ts(q, 2), :]
                        )
                else:
                    nc.gpsimd.dma_start(out=xg, in_=src)
                xgs.append(xg)

            # ---- PE HAM warmup: ~3.8us of sustained matmul activity
            # flips the clock gate to 2.4GHz before the real pipeline.
            warm_ps = ps_mm1.tile([128, 2, 512], f32, tag="mm1")
            for _ in range(_WARM_MMS):
                nc.tensor.matmul(
                    warm_ps[:, 0, :], lhsT=ident_sb, rhs=warm_rhs,
                    start=True, stop=True,
                )
            warmsink = consts.tile([128, 1], f32)
            nc.vector.tensor_copy(out=warmsink, in_=warm_ps[:, 0, 0:1])
            # warm the ACT table set (sqrt anchor; relu/identity ride along)
            warm = consts.tile([128, 1], f32)
            nc.scalar.activation(out=warm, in_=eps, func=AF.Sqrt)
            nc.scalar.activation(out=warm, in_=eps, func=AF.Relu)
            nc.scalar.activation(out=warm, in_=eps, func=AF.Identity, bias=eps)

            for g in range(_PAIRS):
                xg = xgs[g]

                # ---- transpose to (d, l); f16 PSUM ----
                xtp = ps_t.tile([128, _CPP, 128], f16, tag="xtp")
                for c in range(_CPP):
                    nc.tensor.transpose(xtp[:, c, :], xg[:, c, :], ident_sb)
                xT = work.tile([128, 1024], f16, tag="xT")
                nc.vector.tensor_copy(
                    out=xT, in_=xtp.rearrange("p c d -> p (c d)")
                )

                # ---- PFF; residual folded into the mm2 accumulation
                # group as ident.T @ xT ----
                y1p = ps_mm1.tile([128, 2, 512], f32, tag="mm1")
                for k in range(2):
                    nc.tensor.matmul(
                        y1p[:, k, :], lhsT=w1_sb, rhs=xT[:, ts(k, 512)],
                        start=True, stop=True,
                    )
                y1s = work.tile([128, 1024], f16, tag="y1s")
                nc.scalar.activation(
                    out=y1s, in_=y1p.rearrange("p k n -> p (k n)"), func=AF.Relu
                )
                pp = ps_mm2.tile([128, 2, 512], f32, tag="mm2")
                for k in range(2):
                    nc.tensor.matmul(
                        pp[:, k, :], lhsT=w2_sb, rhs=y1s[:, ts(k, 512)],
                        start=True, stop=False,
                    )
                    nc.tensor.matmul(
                        pp[:, k, :], lhsT=ident_sb, rhs=xT[:, ts(k, 512)],
                        start=False, stop=True,
                    )
                y2s = work.tile([128, 1024], f16, tag="y2s")
                nc.scalar.activation(
                    out=y2s, in_=pp.rearrange("p k n -> p (k n)"), func=AF.Identity
                )

                # ---- transpose back to (l, d); f16 PSUM ----
                ppT = ps_tb.tile([128, _CPP, 128], f16, tag="ppT")
                for c in range(_CPP):
                    nc.tensor.transpose(ppT[:, c, :], y2s[:, ts(c, 128)], ident_sb)
                pn = pnp.tile([128, _CPP, 128], f16, tag="pn")
                nc.vector.tensor_copy(
                    out=pn.rearrange("p c d -> p (c d)"),
                    in_=ppT.rearrange("p c d -> p (c d)"),
                )

                # ---- LN stats ----
                bstats = small.tile([128, _CPP, 6], f32, tag="bstats")
                if _BN3D:
                    nc.vector.bn_stats(out=bstats, in_=pn)
                else:
                    for c in range(_CPP):
                        nc.vector.bn_stats(out=bstats[:, c, :], in_=pn[:, c, :])
                # combine even/odd halves (counts equal 64):
                #   mean = (me+mo)/2
                #   var  = (cve+cvo)/128 + ((me-mo)/2)^2
                me, mo = bstats[:, :, 1], bstats[:, :, 4]
                cve, cvo = bstats[:, :, 2], bstats[:, :, 5]
                sm = small.tile([128, _CPP], f32, tag="sm")
                nc.vector.tensor_add(out=sm, in0=me, in1=mo)
                dm = small.tile([128, _CPP], f32, tag="dm")
                nc.vector.tensor_sub(out=dm, in0=me, in1=mo)
                dsq = small.tile([128, _CPP], f32, tag="dsq")
                nc.vector.scalar_tensor_tensor(
                    out=dsq, in0=dm, scalar=0.25, in1=dm,
                    op0=OP.mult, op1=OP.mult,
                )
                vs = small.tile([128, _CPP], f32, tag="vs")
                nc.vector.tensor_add(out=vs, in0=cve, in1=cvo)
                var = small.tile([128, _CPP], f32, tag="var")
                nc.vector.scalar_tensor_tensor(
                    out=var, in0=vs, scalar=1.0 / 128.0, in1=dsq,
                    op0=OP.mult, op1=OP.add,
                )
                std = small.tile([128, _CPP], f32, tag="std")
                nc.scalar.activation(
                    out=std, in_=var, func=AF.Sqrt, scale=1.0, bias=eps
                )
                rstd = small.tile([128, _CPP], f32, tag="rstd")
                nc.vector.reciprocal(out=rstd, in_=std)
                nmr = small.tile([128, _CPP], f32, tag="nmr")
                nc.vector.scalar_tensor_tensor(
                    out=nmr, in0=sm, scalar=-0.5, in1=rstd,
                    op0=OP.mult, op1=OP.mult,
                )

                # ---- apply LN: out = pn*rstd + nmr (f32 out) ----
                og = io.tile([128, _CPP, 128], f32, tag="og")
                for c in range(_CPP):
                    eng = _NORM_ENG[c]
                    if eng == "act":
                        nc.scalar.activation(
                            out=og[:, c, :], in_=pn[:, c, :], func=AF.Identity,
                            bias=nmr[:, c : c + 1], scale=rstd[:, c : c + 1],
                        )
                    else:
                        veng = nc.vector if eng == "dve" else nc.gpsimd
                        veng.tensor_scalar(
                            out=og[:, c, :], in0=pn[:, c, :],
                            scalar1=rstd[:, c : c + 1], scalar2=nmr[:, c : c + 1],
                            op0=OP.mult, op1=OP.add,
                        )

                dst = y[ts(g, 1024), :].rearrange("(p r) d -> p r d", p=128)
                eng = nc.sync if g % 2 == 0 else nc.scalar
                eng.dma_start(out=dst, in_=og)
    nc.finalize()
    return nc


def _ensure_ntff_hook():
    """Register the axon NTFF profiling hook if the image lacks antenv.axon_hooks."""
    try:
        from antenv.axon_hooks import get_axon_ntff_profile_hook  # noqa: F401
        return
    except ImportError:
        pass
    import sys
    import types

    import antenv
    from trn_agent_boot.trn_boot import _ntff_profile_via_ctypes

    hook = _ntff_profile_via_ctypes("/opt/axon/libaxon_pjrt.so")
    mod = types.ModuleType("antenv.axon_hooks")
    mod._hook = hook
    mod.set_axon_ntff_profile_hook = lambda h: setattr(mod, "_hook", h)
    mod.get_axon_ntff_profile_hook = lambda: mod._hook
    sys.modules["antenv.axon_hooks"] = mod
    antenv.axon_hooks = mod


def _run_device(x, w1, w2, trace=False):
    import concourse.bass_utils as bass_utils
    from concourse.bass_utils import run_bass_kernel_spmd

    if trace:
        try:
            _ensure_ntff_hook()
            bass_utils.upload_artifacts = lambda tmpdir: str(tmpdir)
        except Exception as e:  # profiling is best-effort
            print(f"ntff hook unavailable ({e}); running without trace")
            trace = False

    if "prog" not in _prog_cache:
        _prog_cache["prog"] = _build_program()
    nc = _prog_cache["prog"]
    w1h = np.ascontiguousarray(w1, dtype=np.float16)
    w2h = np.ascontiguousarray(w2, dtype=np.float16)
    identh = np.eye(DX, dtype=np.float16)
    in_maps = [
        {
            "x": np.ascontiguousarray(x[b], dtype=np.float32),
            "w1": w1h,
            "w2": w2h,
            "identp": identh,
        }
        for b in range(B)
    ]
    res = run_bass_kernel_spmd(
        nc, in_maps, core_ids=list(range(B)), trace=trace,
        trace_cores=list(range(B)) if trace else None,
    )
    kernel.last_result = res
    kernel.last_exec_time_ns = res.exec_time_ns
    return np.stack([r["y"] for r in res.results], axis=0)


def _numpy_fallback(inputs):
    """Faithful (but slow) mirror of the reference for unexpected inputs."""
    f32 = np.float32
    x = np.asarray(inputs["x"], f32)
    c = np.asarray(inputs["c"], f32)
    W1 = np.asarray(inputs["W1"], f32); W2 = np.asarray(inputs["W2"], f32)
    wt_w = np.asarray(inputs["wt_w"], f32); bsa = np.asarray(inputs["bsa"], f32)
    Wsa1 = np.asarray(inputs["Wsa1"], f32); Wsa2 = np.asarray(inputs["Wsa2"], f32)
    wsat_w = np.asarray(inputs["wsat_w"], f32)
    wsat_b = np.asarray(inputs["wsat_b"], f32); bsa1 = np.asarray(inputs["bsa1"], f32)
    pfn_w1 = np.asarray(inputs["pfn_w1"], f32); pfn_b1 = np.asarray(inputs["pfn_b1"], f32)
    pfn_w2 = np.asarray(inputs["pfn_w2"], f32); pfn_b2 = np.asarray(inputs["pfn_b2"], f32)
    ln_g = np.asarray(inputs["ln_g"], f32); ln_b = np.asarray(inputs["ln_b"], f32)
    Bs, Ls, _ = x.shape
    wx = x @ W1
    wq = c @ W2
    logits = (wx + wq[:, None, :] + bsa) @ wt_w
    m = logits.max(-1, keepdims=True)
    e = np.exp(logits - m)
    p = (e / e.sum(-1, keepdims=True))[..., None]
    h = x * p
    si = (h @ Wsa1) @ wsat_w
    sj = (h @ Wsa2) @ wsat_w
    const = bsa1 @ wsat_w + wsat_b
    colsum = np.zeros((Bs, Ls), f32)
    blk = 512
    for b in range(Bs):
        for i0 in range(0, Ls, blk):
            s = 1.0 / (1.0 + np.exp(-(si[b, i0 : i0 + blk, None] + sj[b, None, :] + const)))
            for r in range(s.shape[0]):
                s[r, i0 + r] = -np.inf
            sm = s.max(-1, keepdims=True)
            ee = np.exp(s - sm)
            colsum[b] += (ee / ee.sum(-1, keepdims=True)).sum(0)
    ui = x * colsum[..., None]
    yv = np.maximum(ui @ pfn_w1 + pfn_b1, 0.0)
    yv = yv @ pfn_w2 + pfn_b2 + ui
    mu = yv.mean(-1, keepdims=True)
    var = ((yv - mu) ** 2).mean(-1, keepdims=True)
    return ((yv - mu) / np.sqrt(var + 1e-6) * ln_g + ln_b).astype(f32)


def kernel(**inputs):
    x = np.asarray(inputs["x"], dtype=np.float32)
    pfn_w1 = np.asarray(inputs["pfn_w1"], dtype=np.float32)
    pfn_w2 = np.asarray(inputs["pfn_w2"], dtype=np.float32)

    fast_ok = (
        x.shape == (B, L, DX)
        and not np.any(np.asarray(inputs["pfn_b1"]))
        and not np.any(np.asarray(inputs["pfn_b2"]))
        and np.all(np.asarray(inputs["ln_g"]) == 1.0)
        and not np.any(np.asarray(inputs["ln_b"]))
    )
    if not fast_ok:
        return _numpy_fallback(inputs)

    trace = bool(int(os.environ.get("CSA_TRACE", "0")))
    return _run_device(x, pfn_w1, pfn_w2, trace=trace)


kernel.last_exec_time_ns = None
kernel.last_result = None


# revision 11
# speedup vs baseline: 1.1519x; 1.0829x over previous
"""Trainium2 Bass kernel for nn_CSATransformer_25778393710760.

Math: with this problem's parameters (all biases zero, ln affine identity),
the attention colsum scale cancels through LayerNorm (PFF is positively
homogeneous, colsum > 0), so
    out = LN(relu(x @ pfn_w1) @ pfn_w2 + x)
to ~1e-8.  Sharding: pure data parallel over batch B=8 across 8 cores.

Per-core kernel (L=4096 rows, D=128), fp16 matmul path (~5e-4 rel err,
gate is 2e-2), processed as 4 pairs of 1024 rows:
  SWDGE cast-DMA in (f32 HBM -> f16 SBUF, 4KB/partition lines; partition
  p holds rows 8p..8p+7) -> 8x PE transpose (f16 PSUM, LDW-bound) ->
  one DVE copy [128,1024] -> 2x w1 matmul -> one ACT relu [128,1024] ->
  2x (w2 matmul + residual as accumulating ident matmul) -> one ACT copy
  -> 8x PE transpose back (f16 PSUM) -> one DVE copy -> bn_stats +
  wide [128,8] stat combines -> sqrt/recip -> normalize split across
  ACT/DVE/GPSIMD -> HWDGE store (4KB lines).
The transpose column permutation induced by the DMA layout cancels
between the in- and out-transposes.  PE HAM warmup: 9 x N=512 matmuls
(~3.8us sustained) flip the clock gate to 2.4GHz before the pipeline.
"""

import os
import numpy as np

B, L, DX = 8, 4096, 128
_PAIRS = 4          # 1024-row blocks per core
_CPP = 8            # 128-row chunks per block
_WARM_MMS = 9       # N=512 PE warmup matmuls (~3.8us cold)
_BN3D = False       # grouped bn_stats crashes walrus (AP flattens); per chunk
# normalize chunk -> engine: 3x ACT, 2x DVE, 3x GPSIMD
_NORM_ENG = ("act", "act", "act", "dve", "dve", "gps", "gps", "gps")

_prog_cache = {}


def _build_program():
    import concourse.tile as tile
    from concourse import bacc, mybir
    from concourse.bass import ts

    f32 = mybir.dt.float32
    f16 = mybir.dt.float16
    AF = mybir.ActivationFunctionType
    OP = mybir.AluOpType

    nc = bacc.Bacc(None, target_bir_lowering=False)
    x = nc.dram_tensor("x", [L, DX], f32, kind="ExternalInput")
    w1 = nc.dram_tensor("w1", [DX, DX], f16, kind="ExternalInput")
    w2 = nc.dram_tensor("w2", [DX, DX], f16, kind="ExternalInput")
    identp = nc.dram_tensor("identp", [DX, DX], f16, kind="ExternalInput")
    y = nc.dram_tensor("y", [L, DX], f32, kind="ExternalOutput")

    with tile.TileContext(nc) as tc:
        with (
            tc.tile_pool(name="consts", bufs=1) as consts,
            tc.tile_pool(name="xg_pool", bufs=_PAIRS) as xg_pool,
            tc.tile_pool(name="work", bufs=2) as work,
            tc.tile_pool(name="pnp", bufs=_PAIRS) as pnp,
            tc.tile_pool(name="small", bufs=_PAIRS) as small,
            tc.tile_pool(name="io", bufs=3) as io,
            tc.tile_pool(name="ps_t", bufs=2, space="PSUM") as ps_t,
            tc.tile_pool(name="ps_mm1", bufs=1, space="PSUM") as ps_mm1,
            tc.tile_pool(name="ps_mm2", bufs=1, space="PSUM") as ps_mm2,
            tc.tile_pool(name="ps_tb", bufs=2, space="PSUM") as ps_tb,
        ):
            # ---- tiny const DMAs first: everything gates on these ----
            ident_sb = consts.tile([128, 128], f16)
            w1_sb = consts.tile([128, 128], f16)
            w2_sb = consts.tile([128, 128], f16)
            nc.sync.dma_start(out=ident_sb, in_=identp[:, :])
            nc.scalar.dma_start(out=w1_sb, in_=w1[:, :])
            nc.scalar.dma_start(out=w2_sb, in_=w2[:, :])
            eps = consts.tile([128, 1], f32)
            nc.vector.memset(eps, 1e-6)
            warm_rhs = consts.tile([128, 512], f16)
            nc.vector.memset(warm_rhs, 0.5)

            # ---- issue all x cast-loads (f32 HBM -> f16 SBUF, SWDGE).
            # pair 0 in quarters so the first transposes start ASAP.
            xgs = []
            for g in range(_PAIRS):
                xg = xg_pool.tile([128, _CPP, 128], f16, tag="xg")
                src = x[ts(g, 1024), :].rearrange("(p r) d -> p r d", p=128)
                if g == 0:
                    for q in range(4):
                        nc.gpsimd.dma_start(
                            out=xg[:, ts(q, 2), :], in_=src[:, ts(q, 2), :]
                        )
                else:
                    nc.gpsimd.dma_start(out=xg, in_=src)
                xgs.append(xg)

            # ---- PE HAM warmup: ~3.8us of sustained matmul activity
            # flips the clock gate to 2.4GHz before the real pipeline.
            warm_ps = ps_mm1.tile([128, 2, 512], f32, tag="mm1")
            for _ in range(_WARM_MMS):
                nc.tensor.matmul(
                    warm_ps[:, 0, :], lhsT=ident_sb, rhs=warm_rhs,
                    start=True, stop=True,
                )
            warmsink = consts.tile([128, 1], f32)
            nc.vector.tensor_copy(out=warmsink, in_=warm_ps[:, 0, 0:1])
            # warm the ACT table set (sqrt anchor; relu/identity ride along)
            warm = consts.tile([128, 1], f32)
            nc.scalar.activation(out=warm, in_=eps, func=AF.Sqrt)
            nc.scalar.activation(out=warm, in_=eps, func=AF.Relu)
            nc.scalar.activation(out=warm, in_=eps, func=AF.Identity, bias=eps)

            # ---- front sweep: all pairs' transpose/matmul/copy work is
            # issued before any stats work, so the strict per-engine
            # FIFOs never block a later pair's front half behind an
            # earlier pair's stats chain.
            pns = []
            for g in range(_PAIRS):
                xg = xgs[g]

                # ---- transpose to (d, l); f16 PSUM ----
                xtp = ps_t.tile([128, _CPP, 128], f16, tag="xtp")
                for c in range(_CPP):
                    nc.tensor.transpose(xtp[:, c, :], xg[:, c, :], ident_sb)
                xT = work.tile([128, 1024], f16, tag="xT")
                nc.vector.tensor_copy(
                    out=xT, in_=xtp.rearrange("p c d -> p (c d)")
                )

                # ---- PFF; residual folded into the mm2 accumulation
                # group as ident.T @ xT ----
                y1p = ps_mm1.tile([128, 2, 512], f32, tag="mm1")
                for k in range(2):
                    nc.tensor.matmul(
                        y1p[:, k, :], lhsT=w1_sb, rhs=xT[:, ts(k, 512)],
                        start=True, stop=True,
                    )
                y1s = work.tile([128, 1024], f16, tag="y1s")
                nc.scalar.activation(
                    out=y1s, in_=y1p.rearrange("p k n -> p (k n)"), func=AF.Relu
                )
                pp = ps_mm2.tile([128, 2, 512], f32, tag="mm2")
                for k in range(2):
                    nc.tensor.matmul(
                        pp[:, k, :], lhsT=w2_sb, rhs=y1s[:, ts(k, 512)],
                        start=True, stop=False,
                    )
                    nc.tensor.matmul(
                        pp[:, k, :], lhsT=ident_sb, rhs=xT[:, ts(k, 512)],
                        start=False, stop=True,
                    )
                y2s = work.tile([128, 1024], f16, tag="y2s")
                nc.scalar.activation(
                    out=y2s, in_=pp.rearrange("p k n -> p (k n)"), func=AF.Identity
                )

                # ---- transpose back to (l, d); f16 PSUM -> SBUF ----
                ppT = ps_tb.tile([128, _CPP, 128], f16, tag="ppT")
                for c in range(_CPP):
                    nc.tensor.transpose(ppT[:, c, :], y2s[:, ts(c, 128)], ident_sb)
                pn = pnp.tile([128, _CPP, 128], f16, tag="pn")
                nc.vector.tensor_copy(
                    out=pn.rearrange("p c d -> p (c d)"),
                    in_=ppT.rearrange("p c d -> p (c d)"),
                )
                pns.append(pn)

            # ---- stats sweep ----
            scales = []
            for g in range(_PAIRS):
                pn = pns[g]
                bstats = small.tile([128, _CPP, 6], f32, tag="bstats")
                if _BN3D:
                    nc.vector.bn_stats(out=bstats, in_=pn)
                else:
                    for c in range(_CPP):
                        nc.vector.bn_stats(out=bstats[:, c, :], in_=pn[:, c, :])
                # combine even/odd halves (counts equal 64):
                #   mean = (me+mo)/2
                #   var  = (cve+cvo)/128 + ((me-mo)/2)^2
                me, mo = bstats[:, :, 1], bstats[:, :, 4]
                cve, cvo = bstats[:, :, 2], bstats[:, :, 5]
                sm = small.tile([128, _CPP], f32, tag="sm")
                nc.vector.tensor_add(out=sm, in0=me, in1=mo)
                dm = small.tile([128, _CPP], f32, tag="dm")
                nc.vector.tensor_sub(out=dm, in0=me, in1=mo)
                dsq = small.tile([128, _CPP], f32, tag="dsq")
                nc.vector.scalar_tensor_tensor(
                    out=dsq, in0=dm, scalar=0.25, in1=dm,
                    op0=OP.mult, op1=OP.mult,
                )
                vs = small.tile([128, _CPP], f32, tag="vs")
                nc.vector.tensor_add(out=vs, in0=cve, in1=cvo)
                var = small.tile([128, _CPP], f32, tag="var")
                nc.vector.scalar_tensor_tensor(
                    out=var, in0=vs, scalar=1.0 / 128.0, in1=dsq,
                    op0=OP.mult, op1=OP.add,
                )
                std = small.tile([128, _CPP], f32, tag="std")
                nc.scalar.activation(
                    out=std, in_=var, func=AF.Sqrt, scale=1.0, bias=eps
                )
                rstd = small.tile([128, _CPP], f32, tag="rstd")
                nc.vector.reciprocal(out=rstd, in_=std)
                nmr = small.tile([128, _CPP], f32, tag="nmr")
                nc.vector.scalar_tensor_tensor(
                    out=nmr, in0=sm, scalar=-0.5, in1=rstd,
                    op0=OP.mult, op1=OP.mult,
                )
                scales.append((rstd, nmr))

            # ---- apply + store sweep ----
            for g in range(_PAIRS):
                pn = pns[g]
                rstd, nmr = scales[g]
                og = io.tile([128, _CPP, 128], f32, tag="og")
                for c in range(_CPP):
                    eng = _NORM_ENG[c]
                    if eng == "act":
                        nc.scalar.activation(
                            out=og[:, c, :], in_=pn[:, c, :], func=AF.Identity,
                            bias=nmr[:, c : c + 1], scale=rstd[:, c : c + 1],
                        )
                    else:
                        veng = nc.vector if eng == "dve" else nc.gpsimd
                        veng.tensor_scalar(
                            out=og[:, c, :], in0=pn[:, c, :],
                            scalar1=rstd[:, c : c + 1], scalar2=nmr[:, c : c + 1],
                            op0=OP.mult, op1=OP.add,
                        )

                dst = y[ts(g, 1024), :].rearrange("(p r) d -> p r d", p=128)
                eng = nc.sync if g % 2 == 0 else nc.scalar
                eng.dma_start(out=dst, in_=og)
    nc.finalize()
    return nc


def _ensure_ntff_hook():
    """Register the axon NTFF profiling hook if the image lacks antenv.axon_hooks."""
    try:
        from antenv.axon_hooks import get_axon_ntff_profile_hook  # noqa: F401
        return
    except ImportError:
        pass
    import sys
    import types

    import antenv
    from trn_agent_boot.trn_boot import _ntff_profile_via_ctypes

    hook = _ntff_profile_via_ctypes("/opt/axon/libaxon_pjrt.so")
    mod = types.ModuleType("antenv.axon_hooks")
    mod._hook = hook
    mod.set_axon_ntff_profile_hook = lambda h: setattr(mod, "_hook", h)
    mod.get_axon_ntff_profile_hook = lambda: mod._hook
    sys.modules["antenv.axon_hooks"] = mod
    antenv.axon_hooks = mod


def _run_device(x, w1, w2, trace=False):
    import concourse.bass_utils as bass_utils
    from concourse.bass_utils import run_bass_kernel_spmd

    if trace:
        try:
            _ensure_ntff_hook()
            bass_utils.upload_artifacts = lambda tmpdir: str(tmpdir)
        except Exception as e:  # profiling is best-effort
            print(f"ntff hook unavailable ({e}); running without trace")
            trace = False

    if "prog" not in _prog_cache:
        _prog_cache["prog"] = _build_program()
    nc = _prog_cache["prog"]
    w1h = np.ascontiguousarray(w1, dtype=np.float16)
    w2h = np.ascontiguousarray(w2, dtype=np.float16)
    identh = np.eye(DX, dtype=np.float16)
    in_maps = [
        {
            "x": np.ascontiguousarray(x[b], dtype=np.float32),
            "w1": w1h,
            "w2": w2h,
            "identp": identh,
        }
        for b in range(B)
    ]
    res = run_bass_kernel_spmd(
        nc, in_maps, core_ids=list(range(B)), trace=trace,
        trace_cores=list(range(B)) if trace else None,
    )
    kernel.last_result = res
    kernel.last_exec_time_ns = res.exec_time_ns
    return np.stack([r["y"] for r in res.results], axis=0)


def _numpy_fallback(inputs):
    """Faithful (but slow) mirror of the reference for unexpected inputs."""
    f32 = np.float32
    x = np.asarray(inputs["x"], f32)
    c = np.asarray(inputs["c"], f32)
    W1 = np.asarray(inputs["W1"], f32); W2 = np.asarray(inputs["W2"], f32)
    wt_w = np.asarray(inputs["wt_w"], f32); bsa = np.asarray(inputs["bsa"], f32)
    Wsa1 = np.asarray(inputs["Wsa1"], f32); Wsa2 = np.asarray(inputs["Wsa2"], f32)
    wsat_w = np.asarray(inputs["wsat_w"], f32)
    wsat_b = np.asarray(inputs["wsat_b"], f32); bsa1 = np.asarray(inputs["bsa1"], f32)
    pfn_w1 = np.asarray(inputs["pfn_w1"], f32); pfn_b1 = np.asarray(inputs["pfn_b1"], f32)
    pfn_w2 = np.asarray(inputs["pfn_w2"], f32); pfn_b2 = np.asarray(inputs["pfn_b2"], f32)
    ln_g = np.asarray(inputs["ln_g"], f32); ln_b = np.asarray(inputs["ln_b"], f32)
    Bs, Ls, _ = x.shape
    wx = x @ W1
    wq = c @ W2
    logits = (wx + wq[:, None, :] + bsa) @ wt_w
    m = logits.max(-1, keepdims=True)
    e = np.exp(logits - m)
    p = (e / e.sum(-1, keepdims=True))[..., None]
    h = x * p
    si = (h @ Wsa1) @ wsat_w
    sj = (h @ Wsa2) @ wsat_w
    const = bsa1 @ wsat_w + wsat_b
    colsum = np.zeros((Bs, Ls), f32)
    blk = 512
    for b in range(Bs):
        for i0 in range(0, Ls, blk):
            s = 1.0 / (1.0 + np.exp(-(si[b, i0 : i0 + blk, None] + sj[b, None, :] + const)))
            for r in range(s.shape[0]):
                s[r, i0 + r] = -np.inf
            sm = s.max(-1, keepdims=True)
            ee = np.exp(s - sm)
            colsum[b] += (ee / ee.sum(-1, keepdims=True)).sum(0)
    ui = x * colsum[..., None]
    yv = np.maximum(ui @ pfn_w1 + pfn_b1, 0.0)
    yv = yv @ pfn_w2 + pfn_b2 + ui
    mu = yv.mean(-1, keepdims=True)
    var = ((yv - mu) ** 2).mean(-1, keepdims=True)
    return ((yv - mu) / np.sqrt(var + 1e-6) * ln_g + ln_b).astype(f32)


def kernel(**inputs):
    x = np.asarray(inputs["x"], dtype=np.float32)
    pfn_w1 = np.asarray(inputs["pfn_w1"], dtype=np.float32)
    pfn_w2 = np.asarray(inputs["pfn_w2"], dtype=np.float32)

    fast_ok = (
        x.shape == (B, L, DX)
        and not np.any(np.asarray(inputs["pfn_b1"]))
        and not np.any(np.asarray(inputs["pfn_b2"]))
        and np.all(np.asarray(inputs["ln_g"]) == 1.0)
        and not np.any(np.asarray(inputs["ln_b"]))
    )
    if not fast_ok:
        return _numpy_fallback(inputs)

    trace = bool(int(os.environ.get("CSA_TRACE", "0")))
    return _run_device(x, pfn_w1, pfn_w2, trace=trace)


kernel.last_exec_time_ns = None
kernel.last_result = None
